# revision 17
# baseline (speedup 1.0000x reference)
"""Trainium2 Bass kernel for DFMN prototypical-network loss (retrieval_knn).

Reference math (per pixel, 64-way episode):
    protos = prototypes[indices]                         # [64, C]
    logits = -(|q|^2 + |p|^2 - 2 q.p)                    # [N, 64]
    loss   = -mean(log_softmax(logits)[label])

Key identity: the per-pixel |q|^2 term is constant across classes, so it
cancels in log_softmax.  With G = q.p and z = 2G - pn (pn = |p|^2 per class):
    -logp[label] = LSE_c(z) - z[label]
    loss = [ sum_px LSE_c(z) - sum_img (2*rowsumG[label_b] - 196*pn[label_b]) ] / N

Device layout per core (64 images, data-parallel over 8 cores).  Work is
organized in 16 "units" of 2 image pairs; the two pairs of a unit occupy the
two partition halves of one PSUM bank via PE column tiling (tile_position
(0,0) / (0,64)), so the exp and reductions run on full 128-partition tiles:
  - G via TensorE:   G[128, 392] = protosT_k.T @ Q_k  (8 K-chunks, fp8e4,
    col-tiled pair of matmuls per unit; prototype weights loaded once per
    K-chunk per group and redundant LDWEIGHTS deduplicated)
  - exp via ScalarE: e = Exp(2*G - pn)  (PSUM -> SBUF bf16, fused scale/bias)
  - colsum via TensorE: s[32, 392] += selector.T @ e  (selector has a ones
    column per partition half; two accumulation chains A=units 0..13 /
    B=units 14,15 so the bulk Ln can run before the tail unit finishes)
  - label rowsums via VectorE: r2[128, 32] = per-image free-dim reduce of G
  - final ScalarE Ln x2 with accum_out -> per-row sums of log s, written
    into column 32 of the rsum output (no separate tiny lse DMA)
Host finishes: label gather from r2, pn terms, exact float64 mean.

Streaming dtype is fp8e4 (TRN e4m3, max +-240): quarters the fp32 HBM
traffic (12.85 MB/core at ~358 GB/s is the roofline) and quantization error
lands ~5e-4 relative on the loss (gate is 2e-2).  The q DRAM layout is
partition-major [128, NPAIR*QCOLS] so every group DMA is one contiguous run
per partition (128 fat descriptors instead of ~770 thin ones).

Both Exp and Ln resolve to the `natural_log_exp_and_others` activation
table set (it contains both), so the kernel performs exactly one
ACT_TABLE_LOAD — the baseline paid 4 switches of ~1.3us, one on the tail.
"""

import sys

for _p in ("/opt/trn_rl_repo",):
    if _p not in sys.path:
        sys.path.insert(0, _p)

import numpy as np

import bass_rust as _bass_rust
import concourse.bass as bass
import concourse.bacc as bacc
import concourse.tile as tile
from concourse import mybir
from concourse.bass_utils import run_bass_kernel_spmd
from concourse.hw_specs import get_activation_tables

# Problem constants (nn_DFMNLoss: B=512, C=1024, 14x14 features, 64-way)
B = 512
C = 1024
F2 = 196          # 14 * 14 pixels per image
NWAY = 64
NCORES = 8
BPC = B // NCORES           # 64 images per core
NPAIR = BPC // 2            # 32 image pairs per core
NU = NPAIR // 2             # 16 col-tiled units (2 pairs each)
KT = C // 128               # 8 contraction chunks of 128 channels
PAIRCOLS = 2 * F2           # 392 pixel columns per pair
QCOLS = KT * PAIRCOLS       # 3136 fp8 elements per partition per pair
HK = (KT // 2) * PAIRCOLS   # 1568: half-k block of a pair

# Group plan: leading singleton starts the PE ~12us earlier than the
# baseline's 3-unit first group; singletons are DMA'd in per-pair-half
# quarters so the k<4 matmuls overlap the later quarters' arrival.
PLAN = [[0], [1, 2], [3, 4, 5], [6, 7, 8], [9, 10, 11], [12, 13, 14], [15]]
CHAIN_A_LAST = 13           # s-accumulation chain split: A=0..13, B=14,15

F32 = mybir.dt.float32
F16 = mybir.dt.float16
BF16 = mybir.dt.bfloat16
F8 = mybir.dt.float8e4
F8_NP = mybir.dt.np(F8)
BF16_NP = mybir.dt.np(BF16)

_CACHE = {}


class _Bacc(bacc.Bacc):
    def insert_act_table_loads(self):
        """Same pass as Bacc, but Exp/Ln are removed from every activation
        table set except `natural_log_exp_and_others` (which genuinely
        contains both), so the fixpoint places a single load for the whole
        kernel instead of thrashing exp_and_others <-> natural_log."""
        has_activation = any(
            isinstance(i, mybir.InstActivation)
            for b in self.main_func.blocks
            for i in b.instructions
        )
        if not has_activation:
            return
        exp_ln = {
            mybir.ActivationFunctionType.Exp,
            mybir.ActivationFunctionType.Ln,
            mybir.ActivationFunctionType.Copy,
        }
        tables = [
            (name, fns if name == "natural_log_exp_and_others" else fns - exp_ln)
            for name, fns in get_activation_tables(self.m.arch).items()
        ]
        _bass_rust.insert_act_table_loads(self, tables)


def _dedup_ldweights(nc):
    """Drop InstLdweights that reload weights already resident in the PE
    array.  Tile emits one LDWEIGHTS per matmul; inside a group the same
    prototype chunk is loaded for every unit, and the loads serialize with
    the matmul stream on the PE.  Matmults here are non-self-loading
    (ldweights=False), so a dropped reload just reuses the array contents.
    Tracks state per 32-wide PE column group; only sync-free LDWs are
    dropped, and any dangling dependency names are remapped to the keeper.
    """
    removed = {}
    for blk in nc.m.functions[0].blocks:
        state = {}  # col_group -> (key, keeper_name)
        kept = []
        for inst in blk.instructions:
            if isinstance(inst, mybir.InstLdweights):
                a = inst.ins[0]
                tp = inst.tile_position or (0, 0)
                ts = inst.tile_size or (128, 128)
                key = (a.memref, a.offset, str(a.ap), str(a.dtype), tp, ts)
                cgs = range(tp[1] // 32, (tp[1] + ts[1] + 31) // 32)
                si = inst.sync_info
                clean = si is None or (not si.on_wait and not si.on_update)
                prev = [state.get(cg) for cg in cgs]
                if clean and all(p is not None and p[0] == key for p in prev):
                    removed[inst.name] = prev[0][1]
                    continue
                for cg in cgs:
                    state[cg] = (key, inst.name)
            kept.append(inst)
        blk.instructions[:] = kept
    if removed:
        for blk in nc.m.functions[0].blocks:
            for inst in blk.instructions:
                names = set(inst.sync_dependency_names()) | set(
                    inst.nosync_dependency_names()
                )
                if names & removed.keys():
                    inst.remap_dependency_names(
                        {k: v for k, v in removed.items() if k in names}
                    )
        for k in removed:
            nc.inst_map.pop(k, None)
    return len(removed)


def _build_nc():
    # Bacc (not raw Bass): its compile() pass splits multi-wait instructions
    # into event semaphores — walrus allows only one sync wait per instruction.
    nc = _Bacc()
    q = nc.dram_tensor("q", [128, NPAIR * QCOLS], F8, kind="ExternalInput")
    pT = nc.dram_tensor("pT", [128, KT * NWAY], F8, kind="ExternalInput")
    negpn2 = nc.dram_tensor("negpn2", [128, 1], F32, kind="ExternalInput")
    bsel2 = nc.dram_tensor("bsel2", [128, 2 * NPAIR - 1], BF16, kind="ExternalInput")
    # rsum[:, 0:32] = per-image G rowsums; rsum[0:28, 32] = chain-A Ln accums,
    # rsum[0:4, 33] = chain-B Ln accums (together the lse total).
    rsum = nc.dram_tensor("rsum", [128, NPAIR + 2], F32, kind="ExternalOutput")

    with tile.TileContext(nc) as tc:
        with (
            tc.tile_pool(name="const", bufs=1) as cpool,
            tc.tile_pool(name="qin", bufs=4) as qpool,
            tc.tile_pool(name="qtail", bufs=1) as tpool,
            tc.tile_pool(name="acc", bufs=1) as apool,
            tc.tile_pool(name="gps", bufs=6, space="PSUM") as gpool,
            tc.tile_pool(name="sps", bufs=1, space="PSUM") as spool,
        ):
            # Singleton groups stream in per-pair-half quarters so the first
            # matmul waits on 0.2 MB, not a whole group.  The constants go
            # through the ACT HW-DGE ring (nc.scalar) so they drain
            # immediately instead of queueing behind the q stream on the SP
            # ring — the first LDWEIGHTS needs pT.
            quarters = {}

            def singleton_dma(u):
                pa, pb = 2 * u, 2 * u + 1
                tiles = []
                for nm, pr, k0 in (
                    ("qa1", pa, 0), ("qb1", pb, 0), ("qa2", pa, HK), ("qb2", pb, HK)
                ):
                    t = tpool.tile([128, HK], F8, name=nm, tag=f"{nm}_{u}")
                    nc.sync.dma_start(
                        t[:], q[:, pr * QCOLS + k0 : pr * QCOLS + k0 + HK]
                    )
                    tiles.append(t)
                quarters[u] = tiles

            p_sb = cpool.tile([128, KT * NWAY], F8)
            nc.scalar.dma_start(p_sb[:], pT[:])
            npn_sb = cpool.tile([128, 1], F32)
            nc.scalar.dma_start(npn_sb[:], negpn2[:])
            bsel_sb = cpool.tile([128, 2 * NPAIR - 1], BF16)
            nc.scalar.dma_start(bsel_sb[:], bsel2[:])

            singleton_dma(PLAN[0][0])

            r_sb = apool.tile([128, NPAIR + 2], F32)
            ltmp = apool.tile([NPAIR, PAIRCOLS], F32)
            e_all = apool.tile([128, NU * PAIRCOLS], BF16)
            sA = spool.tile([NPAIR, PAIRCOLS], F32, name="sA", tag="sA")
            sB = spool.tile([NPAIR, PAIRCOLS], F32, name="sB", tag="sB")

            # ACT warmup: absorb the npn DMA wait, const-AP init and the
            # single exp+ln table load outside the hot loop.
            warm_a = cpool.tile([128, 1], F32)
            warm_b = cpool.tile([128, 1], F32)
            nc.scalar.copy(warm_a[:], npn_sb[:])
            nc.scalar.activation(
                warm_b[:], warm_a[:], mybir.ActivationFunctionType.Exp
            )

            def sel_matmul(u):
                # chain A: unit u -> rows 2u, 2u+1 (units 0..13, rows 0..27);
                # chain B: units 14,15 -> rows 0..3 of sB (activations must
                # start on a 32-aligned partition, so B can't use rows 28..31)
                # row j:  s[j, :] += colsum of the partition half of e(u)
                s_ps = sA if u <= CHAIN_A_LAST else sB
                j0 = 2 * u if u <= CHAIN_A_LAST else 2 * (u - CHAIN_A_LAST - 1)
                nc.tensor.matmul(
                    s_ps[:],
                    bsel_sb[:, NPAIR - 1 - j0 : 2 * NPAIR - 1 - j0],
                    e_all[:, u * PAIRCOLS : (u + 1) * PAIRCOLS],
                    start=(u == 0 or u == CHAIN_A_LAST + 1),
                    stop=(u == CHAIN_A_LAST or u == NU - 1),
                    skip_group_check=True,
                )

            for gi, units in enumerate(PLAN):
                single = len(units) == 1
                gp = 2 * len(units)          # pairs in this group
                p0 = 2 * units[0]            # first pair index
                if single:
                    if units[0] not in quarters:
                        singleton_dma(units[0])
                else:
                    # One group tile, but per-unit DMA slices: a unit's
                    # matmuls start as soon as its own 0.8 MB lands
                    # (subtile deps), not the whole group.
                    gt = qpool.tile([128, gp * QCOLS], F8, name="gt", tag="gt")
                    for jloc in range(len(units)):
                        nc.sync.dma_start(
                            gt[:, 2 * jloc * QCOLS : 2 * (jloc + 1) * QCOLS],
                            q[
                                :,
                                (p0 + 2 * jloc) * QCOLS : (p0 + 2 * jloc + 2) * QCOLS,
                            ],
                        )
                gps = {
                    u: gpool.tile([128, PAIRCOLS], F32, name="gps", tag="gps")
                    for u in units
                }
                for k in range(KT):
                    wk = p_sb[:, k * NWAY : (k + 1) * NWAY]
                    for jloc, u in enumerate(units):
                        if single:
                            ta, tb, ca, cb = (
                                quarters[u][0], quarters[u][1],
                                (k % 4) * PAIRCOLS, (k % 4) * PAIRCOLS,
                            ) if k < 4 else (
                                quarters[u][2], quarters[u][3],
                                (k - 4) * PAIRCOLS, (k - 4) * PAIRCOLS,
                            )
                            srca, srcb = ta, tb
                        else:
                            srca = srcb = gt
                            ca = 2 * jloc * QCOLS + k * PAIRCOLS
                            cb = (2 * jloc + 1) * QCOLS + k * PAIRCOLS
                        nc.tensor.matmul(
                            gps[u][0:NWAY, :],
                            wk,
                            srca[:, ca : ca + PAIRCOLS],
                            tile_position=(0, 0),
                            start=(k == 0),
                            stop=(k == KT - 1),
                            skip_group_check=True,
                        )
                        nc.tensor.matmul(
                            gps[u][NWAY:128, :],
                            wk,
                            srcb[:, cb : cb + PAIRCOLS],
                            tile_position=(0, NWAY),
                            start=(k == 0),
                            stop=(k == KT - 1),
                            skip_group_check=True,
                        )
                # Selector matmuls lag one group so the PE never stalls on
                # the ACT exp (exp(g-1) ran during this group's matmuls).
                if gi > 0:
                    for u in PLAN[gi - 1]:
                        sel_matmul(u)
                for u in units:
                    nc.scalar.activation(
                        e_all[:, u * PAIRCOLS : (u + 1) * PAIRCOLS],
                        gps[u][:],
                        mybir.ActivationFunctionType.Exp,
                        bias=npn_sb[:],
                        scale=2.0,
                    )
                    nc.vector.reduce_sum(
                        r_sb[:, 2 * u : 2 * u + 1],
                        gps[u][:, 0:F2],
                        axis=mybir.AxisListType.X,
                    )
                    nc.vector.reduce_sum(
                        r_sb[:, 2 * u + 1 : 2 * u + 2],
                        gps[u][:, F2:PAIRCOLS],
                        axis=mybir.AxisListType.X,
                    )
            # Chain A is complete (sel(13) issued during the tail group):
            # the bulk Ln runs on ScalarE while the PE does sel(15).
            nc.scalar.activation(
                ltmp[0 : 2 * (CHAIN_A_LAST + 1), :],
                sA[0 : 2 * (CHAIN_A_LAST + 1), :],
                mybir.ActivationFunctionType.Ln,
                accum_out=r_sb[0 : 2 * (CHAIN_A_LAST + 1), NPAIR : NPAIR + 1],
            )
            # Bulk of the label-rowsum output can ship while the tail unit
            # finishes (columns 0..29 are final before the last unit).
            nc.scalar.dma_start(
                rsum[:, 0 : 2 * (NU - 1)], r_sb[:, 0 : 2 * (NU - 1)]
            )
            sel_matmul(NU - 1)
            nb = 2 * (NU - 1 - CHAIN_A_LAST)
            nc.scalar.activation(
                ltmp[0:nb, :],
                sB[0:nb, :],
                mybir.ActivationFunctionType.Ln,
                accum_out=r_sb[0:nb, NPAIR + 1 : NPAIR + 2],
            )
            nc.scalar.dma_start(
                rsum[:, 2 * (NU - 1) : NPAIR + 2],
                r_sb[:, 2 * (NU - 1) : NPAIR + 2],
            )

    n = _dedup_ldweights(nc)
    if n < 64:
        print(f"[kernel] warning: ldweights dedup removed only {n}", flush=True)
    nc.compile()
    return nc


def _get_nc():
    if "nc" not in _CACHE:
        _CACHE["nc"] = _build_nc()
    return _CACHE["nc"]


def _pack_core_q(qc8):
    # fp8 [64, C, F2] -> [p, pair, k, i, f] -> [128, NPAIR*QCOLS]
    qc = qc8.reshape(NPAIR, 2, KT, 128, F2).transpose(3, 0, 2, 1, 4)
    return np.ascontiguousarray(qc).reshape(128, NPAIR * QCOLS)


def _prepare(query_features, labels, prototypes, indices):
    """Returns (in_maps, labels_i64, pn64)."""
    qf = np.asarray(query_features, dtype=np.float32).reshape(B, C, F2)
    labels = np.asarray(labels).astype(np.int64)
    protos = np.asarray(prototypes, dtype=np.float32)
    idx = np.asarray(indices).astype(np.int64)

    pg8 = protos[idx].astype(F8_NP)                      # [64, C] fp8
    pg = pg8.astype(np.float64)
    pn64 = np.sum(pg**2, axis=1)                         # matches device G
    negpn2_np = np.ascontiguousarray(
        np.concatenate([-pn64, -pn64]).reshape(128, 1).astype(np.float32)
    )
    pT_pack = np.ascontiguousarray(
        pg8.T.reshape(KT, 128, NWAY).transpose(1, 0, 2)
    ).reshape(128, KT * NWAY)
    bsel2_np = np.zeros((128, 2 * NPAIR - 1), dtype=BF16_NP)
    bsel2_np[0:NWAY, NPAIR - 1] = 1
    bsel2_np[NWAY:128, NPAIR] = 1

    qf8 = qf.astype(F8_NP)
    in_maps = [
        {
            "q": _pack_core_q(qf8[c * BPC : (c + 1) * BPC]),
            "pT": pT_pack,
            "negpn2": negpn2_np,
            "bsel2": bsel2_np,
        }
        for c in range(NCORES)
    ]
    return in_maps, labels, pn64


def kernel(query_features, labels, prototypes, indices, n_way):
    import time as _time

    t0 = _time.time()
    nc = _get_nc()
    t1 = _time.time()
    in_maps, labels, pn64 = _prepare(query_features, labels, prototypes, indices)
    t2 = _time.time()
    results = run_bass_kernel_spmd(nc, in_maps, list(range(NCORES))).results
    t3 = _time.time()
    print(
        f"[kernel] build={t1 - t0:.1f}s pack={t2 - t1:.1f}s run={t3 - t2:.1f}s",
        flush=True,
    )

    # Host-side finish: rsum[:, 0:32] holds per-image rowsums of G; image
    # local index l lives at row block 64*(l%4>=2)+class, column 2*(l//4)+(l%2).
    # rsum[0:28, 32] + rsum[0:4, 33] hold the per-pair-row sums of log s.
    larr = np.arange(BPC)
    rows0 = 64 * ((larr % 4) >= 2)
    cols = 2 * (larr // 4) + (larr % 2)
    total_lse = 0.0
    label_term = 0.0
    for c in range(NCORES):
        out = results[c]["rsum"].astype(np.float64)      # [128, 34]
        total_lse += float(
            out[0 : 2 * (CHAIN_A_LAST + 1), NPAIR].sum()
            + out[0 : 2 * (NU - 1 - CHAIN_A_LAST), NPAIR + 1].sum()
        )
        r2 = out[:, 0:NPAIR]
        lab = labels[c * BPC : (c + 1) * BPC]
        label_term += float(
            np.sum(2.0 * r2[rows0 + lab, cols] - F2 * pn64[lab])
        )
    loss = (total_lse - label_term) / (B * F2)
    return np.asarray(loss, dtype=np.float32)


# revision 19
# speedup vs baseline: 1.0356x; 1.0356x over previous
"""Trainium2 Bass kernel for DFMN prototypical-network loss (retrieval_knn).

Reference math (per pixel, 64-way episode):
    protos = prototypes[indices]                         # [64, C]
    logits = -(|q|^2 + |p|^2 - 2 q.p)                    # [N, 64]
    loss   = -mean(log_softmax(logits)[label])

Key identity: the per-pixel |q|^2 term is constant across classes, so it
cancels in log_softmax.  With G = q.p and z = 2G - pn (pn = |p|^2 per class):
    -logp[label] = LSE_c(z) - z[label]
    loss = [ sum_px LSE_c(z) - sum_img (2*rowsumG[label_b] - 196*pn[label_b]) ] / N

Device layout per core (64 images, data-parallel over 8 cores).  Work is
organized in 16 "units" of 2 image pairs; the two pairs of a unit occupy the
two partition halves of one PSUM bank via PE column tiling (tile_position
(0,0) / (0,64)), so the exp and reductions run on full 128-partition tiles:
  - G via TensorE:   G[128, 392] = protosT_k.T @ Q_k  (8 K-chunks, fp8e4,
    col-tiled pair of matmuls per unit; prototype weights loaded once per
    K-chunk per group and redundant LDWEIGHTS deduplicated)
  - exp via ScalarE: e = Exp(2*G - pn)  (PSUM -> SBUF bf16, fused scale/bias)
  - colsum via TensorE: s[32, 392] += selector.T @ e  (selector has a ones
    column per partition half; two accumulation chains A=units 0..13 /
    B=units 14,15 so the bulk Ln can run before the tail unit finishes)
  - label rowsums via VectorE: r2[128, 32] = per-image free-dim reduce of G
  - final ScalarE Ln x2 with accum_out -> per-row sums of log s, written
    into column 32 of the rsum output (no separate tiny lse DMA)
Host finishes: label gather from r2, pn terms, exact float64 mean.

Streaming dtype is fp8e4 (TRN e4m3, max +-240): quarters the fp32 HBM
traffic (12.85 MB/core at ~358 GB/s is the roofline) and quantization error
lands ~5e-4 relative on the loss (gate is 2e-2).  The q DRAM layout is
partition-major [128, NPAIR*QCOLS] so every group DMA is one contiguous run
per partition (128 fat descriptors instead of ~770 thin ones).

Both Exp and Ln resolve to the `natural_log_exp_and_others` activation
table set (it contains both), so the kernel performs exactly one
ACT_TABLE_LOAD — the baseline paid 4 switches of ~1.3us, one on the tail.
"""

import sys

for _p in ("/opt/trn_rl_repo",):
    if _p not in sys.path:
        sys.path.insert(0, _p)

import numpy as np

import bass_rust as _bass_rust
import concourse.bass as bass
import concourse.bacc as bacc
import concourse.tile as tile
from concourse import mybir
from concourse.bass_utils import run_bass_kernel_spmd
from concourse.hw_specs import get_activation_tables

# Problem constants (nn_DFMNLoss: B=512, C=1024, 14x14 features, 64-way)
B = 512
C = 1024
F2 = 196          # 14 * 14 pixels per image
NWAY = 64
NCORES = 8
BPC = B // NCORES           # 64 images per core
NPAIR = BPC // 2            # 32 image pairs per core
NU = NPAIR // 2             # 16 col-tiled units (2 pairs each)
KT = C // 128               # 8 contraction chunks of 128 channels
PAIRCOLS = 2 * F2           # 392 pixel columns per pair
QCOLS = KT * PAIRCOLS       # 3136 fp8 elements per partition per pair
HK = (KT // 2) * PAIRCOLS   # 1568: half-k block of a pair

# Group plan: leading singleton starts the PE early (its quarters are
# 0.2 MB); 2-unit groups keep each group's DMA drain (1.6 MB ~ 4.5us)
# under the PE's per-group work so the PE never waits at group boundaries.
PLAN = [[0], [1, 2], [3, 4], [5, 6], [7, 8], [9, 10],
        [11, 12], [13, 14], [15]]
CHAIN_A_LAST = 13           # s-accumulation chain split: A=0..13, B=14,15

F32 = mybir.dt.float32
F16 = mybir.dt.float16
BF16 = mybir.dt.bfloat16
F8 = mybir.dt.float8e4
F8_NP = mybir.dt.np(F8)
BF16_NP = mybir.dt.np(BF16)

_CACHE = {}


class _Bacc(bacc.Bacc):
    def insert_act_table_loads(self):
        """Same pass as Bacc, but Exp/Ln are removed from every activation
        table set except `natural_log_exp_and_others` (which genuinely
        contains both), so the fixpoint places a single load for the whole
        kernel instead of thrashing exp_and_others <-> natural_log."""
        has_activation = any(
            isinstance(i, mybir.InstActivation)
            for b in self.main_func.blocks
            for i in b.instructions
        )
        if not has_activation:
            return
        exp_ln = {
            mybir.ActivationFunctionType.Exp,
            mybir.ActivationFunctionType.Ln,
            mybir.ActivationFunctionType.Copy,
        }
        tables = [
            (name, fns if name == "natural_log_exp_and_others" else fns - exp_ln)
            for name, fns in get_activation_tables(self.m.arch).items()
        ]
        _bass_rust.insert_act_table_loads(self, tables)


def _dedup_ldweights(nc):
    """Drop InstLdweights that reload weights already resident in the PE
    array.  Tile emits one LDWEIGHTS per matmul; inside a group the same
    prototype chunk is loaded for every unit, and the loads serialize with
    the matmul stream on the PE.  Matmults here are non-self-loading
    (ldweights=False), so a dropped reload just reuses the array contents.
    Tracks state per 32-wide PE column group; only sync-free LDWs are
    dropped, and any dangling dependency names are remapped to the keeper.
    """
    removed = {}
    for blk in nc.m.functions[0].blocks:
        state = {}  # col_group -> (key, keeper_name)
        kept = []
        for inst in blk.instructions:
            if isinstance(inst, mybir.InstLdweights):
                a = inst.ins[0]
                tp = inst.tile_position or (0, 0)
                ts = inst.tile_size or (128, 128)
                key = (a.memref, a.offset, str(a.ap), str(a.dtype), tp, ts)
                cgs = range(tp[1] // 32, (tp[1] + ts[1] + 31) // 32)
                si = inst.sync_info
                clean = si is None or (not si.on_wait and not si.on_update)
                prev = [state.get(cg) for cg in cgs]
                if clean and all(p is not None and p[0] == key for p in prev):
                    removed[inst.name] = prev[0][1]
                    continue
                for cg in cgs:
                    state[cg] = (key, inst.name)
            kept.append(inst)
        blk.instructions[:] = kept
    if removed:
        for blk in nc.m.functions[0].blocks:
            for inst in blk.instructions:
                names = set(inst.sync_dependency_names()) | set(
                    inst.nosync_dependency_names()
                )
                if names & removed.keys():
                    inst.remap_dependency_names(
                        {k: v for k, v in removed.items() if k in names}
                    )
        for k in removed:
            nc.inst_map.pop(k, None)
    return len(removed)


def _build_nc():
    # Bacc (not raw Bass): its compile() pass splits multi-wait instructions
    # into event semaphores — walrus allows only one sync wait per instruction.
    nc = _Bacc()
    q = nc.dram_tensor("q", [128, NPAIR * QCOLS], F8, kind="ExternalInput")
    pT = nc.dram_tensor("pT", [128, KT * NWAY], F8, kind="ExternalInput")
    negpn2 = nc.dram_tensor("negpn2", [128, 1], F32, kind="ExternalInput")
    bsel2 = nc.dram_tensor("bsel2", [128, 2 * NPAIR - 1], BF16, kind="ExternalInput")
    # rsum[:, 0:32] = per-image G rowsums; rsum[0:28, 32] = chain-A Ln accums,
    # rsum[0:4, 33] = chain-B Ln accums (together the lse total).
    rsum = nc.dram_tensor("rsum", [128, NPAIR + 2], F32, kind="ExternalOutput")

    with tile.TileContext(nc) as tc:
        with (
            tc.tile_pool(name="const", bufs=1) as cpool,
            tc.tile_pool(name="qin", bufs=4) as qpool,
            tc.tile_pool(name="qtail", bufs=1) as tpool,
            tc.tile_pool(name="acc", bufs=1) as apool,
            tc.tile_pool(name="gps", bufs=6, space="PSUM") as gpool,
            tc.tile_pool(name="sps", bufs=1, space="PSUM") as spool,
        ):
            # Singleton groups stream in per-pair-half quarters so the first
            # matmul waits on 0.2 MB, not a whole group.  The constants go
            # through the ACT HW-DGE ring (nc.scalar) so they drain
            # immediately instead of queueing behind the q stream on the SP
            # ring — the first LDWEIGHTS needs pT.
            quarters = {}

            def singleton_dma(u):
                pa, pb = 2 * u, 2 * u + 1
                tiles = []
                for nm, pr, k0 in (
                    ("qa1", pa, 0), ("qb1", pb, 0), ("qa2", pa, HK), ("qb2", pb, HK)
                ):
                    t = tpool.tile([128, HK], F8, name=nm, tag=f"{nm}_{u}")
                    nc.sync.dma_start(
                        t[:], q[:, pr * QCOLS + k0 : pr * QCOLS + k0 + HK]
                    )
                    tiles.append(t)
                quarters[u] = tiles

            p_sb = cpool.tile([128, KT * NWAY], F8)
            nc.scalar.dma_start(p_sb[:], pT[:])
            npn_sb = cpool.tile([128, 1], F32)
            nc.scalar.dma_start(npn_sb[:], negpn2[:])
            bsel_sb = cpool.tile([128, 2 * NPAIR - 1], BF16)
            nc.scalar.dma_start(bsel_sb[:], bsel2[:])

            singleton_dma(PLAN[0][0])

            r_sb = apool.tile([128, NPAIR + 2], F32)
            ltmp = apool.tile([NPAIR, PAIRCOLS], F32)
            e_all = apool.tile([128, NU * PAIRCOLS], BF16)
            sA = spool.tile([NPAIR, PAIRCOLS], F32, name="sA", tag="sA")
            sB = spool.tile([NPAIR, PAIRCOLS], F32, name="sB", tag="sB")

            # ACT warmup: absorb the npn DMA wait, const-AP init and the
            # single exp+ln table load outside the hot loop.
            warm_a = cpool.tile([128, 1], F32)
            warm_b = cpool.tile([128, 1], F32)
            nc.scalar.copy(warm_a[:], npn_sb[:])
            nc.scalar.activation(
                warm_b[:], warm_a[:], mybir.ActivationFunctionType.Exp
            )

            def sel_matmul(u):
                # chain A: unit u -> rows 2u, 2u+1 (units 0..13, rows 0..27);
                # chain B: units 14,15 -> rows 0..3 of sB (activations must
                # start on a 32-aligned partition, so B can't use rows 28..31)
                # row j:  s[j, :] += colsum of the partition half of e(u)
                s_ps = sA if u <= CHAIN_A_LAST else sB
                j0 = 2 * u if u <= CHAIN_A_LAST else 2 * (u - CHAIN_A_LAST - 1)
                nc.tensor.matmul(
                    s_ps[:],
                    bsel_sb[:, NPAIR - 1 - j0 : 2 * NPAIR - 1 - j0],
                    e_all[:, u * PAIRCOLS : (u + 1) * PAIRCOLS],
                    start=(u == 0 or u == CHAIN_A_LAST + 1),
                    stop=(u == CHAIN_A_LAST or u == NU - 1),
                    skip_group_check=True,
                )

            for gi, units in enumerate(PLAN):
                single = len(units) == 1
                gp = 2 * len(units)          # pairs in this group
                p0 = 2 * units[0]            # first pair index
                if single:
                    if units[0] not in quarters:
                        singleton_dma(units[0])
                else:
                    # One contiguous-per-partition DMA per group.  (Per-unit
                    # DMA slices into a live tile were tried and slowed the
                    # matmul stream ~1.6x via SBUF write/read contention.)
                    gt = qpool.tile([128, gp * QCOLS], F8, name="gt", tag="gt")
                    nc.sync.dma_start(
                        gt[:], q[:, p0 * QCOLS : (p0 + gp) * QCOLS]
                    )
                gps = {
                    u: gpool.tile([128, PAIRCOLS], F32, name="gps", tag="gps")
                    for u in units
                }
                for k in range(KT):
                    wk = p_sb[:, k * NWAY : (k + 1) * NWAY]
                    for jloc, u in enumerate(units):
                        if single:
                            ta, tb, ca, cb = (
                                quarters[u][0], quarters[u][1],
                                (k % 4) * PAIRCOLS, (k % 4) * PAIRCOLS,
                            ) if k < 4 else (
                                quarters[u][2], quarters[u][3],
                                (k - 4) * PAIRCOLS, (k - 4) * PAIRCOLS,
                            )
                            srca, srcb = ta, tb
                        else:
                            srca = srcb = gt
                            ca = 2 * jloc * QCOLS + k * PAIRCOLS
                            cb = (2 * jloc + 1) * QCOLS + k * PAIRCOLS
                        nc.tensor.matmul(
                            gps[u][0:NWAY, :],
                            wk,
                            srca[:, ca : ca + PAIRCOLS],
                            tile_position=(0, 0),
                            start=(k == 0),
                            stop=(k == KT - 1),
                            skip_group_check=True,
                        )
                        nc.tensor.matmul(
                            gps[u][NWAY:128, :],
                            wk,
                            srcb[:, cb : cb + PAIRCOLS],
                            tile_position=(0, NWAY),
                            start=(k == 0),
                            stop=(k == KT - 1),
                            skip_group_check=True,
                        )
                # Selector matmuls lag one group so the PE never stalls on
                # the ACT exp (exp(g-1) ran during this group's matmuls).
                if gi > 0:
                    for u in PLAN[gi - 1]:
                        sel_matmul(u)
                for u in units:
                    nc.scalar.activation(
                        e_all[:, u * PAIRCOLS : (u + 1) * PAIRCOLS],
                        gps[u][:],
                        mybir.ActivationFunctionType.Exp,
                        bias=npn_sb[:],
                        scale=2.0,
                    )
                    nc.vector.reduce_sum(
                        r_sb[:, 2 * u : 2 * u + 1],
                        gps[u][:, 0:F2],
                        axis=mybir.AxisListType.X,
                    )
                    nc.vector.reduce_sum(
                        r_sb[:, 2 * u + 1 : 2 * u + 2],
                        gps[u][:, F2:PAIRCOLS],
                        axis=mybir.AxisListType.X,
                    )
            # Chain A is complete (sel(13) issued during the tail group):
            # the bulk Ln runs on ScalarE while the PE does sel(15).
            nc.scalar.activation(
                ltmp[0 : 2 * (CHAIN_A_LAST + 1), :],
                sA[0 : 2 * (CHAIN_A_LAST + 1), :],
                mybir.ActivationFunctionType.Ln,
                accum_out=r_sb[0 : 2 * (CHAIN_A_LAST + 1), NPAIR : NPAIR + 1],
            )
            # Bulk of the label-rowsum output can ship while the tail unit
            # finishes (columns 0..29 are final before the last unit).
            nc.scalar.dma_start(
                rsum[:, 0 : 2 * (NU - 1)], r_sb[:, 0 : 2 * (NU - 1)]
            )
            sel_matmul(NU - 1)
            nb = 2 * (NU - 1 - CHAIN_A_LAST)
            nc.scalar.activation(
                ltmp[0:nb, :],
                sB[0:nb, :],
                mybir.ActivationFunctionType.Ln,
                accum_out=r_sb[0:nb, NPAIR + 1 : NPAIR + 2],
            )
            nc.scalar.dma_start(
                rsum[:, 2 * (NU - 1) : NPAIR + 2],
                r_sb[:, 2 * (NU - 1) : NPAIR + 2],
            )

    n = _dedup_ldweights(nc)
    if n < 64:
        print(f"[kernel] warning: ldweights dedup removed only {n}", flush=True)
    nc.compile()
    return nc


def _get_nc():
    if "nc" not in _CACHE:
        _CACHE["nc"] = _build_nc()
    return _CACHE["nc"]


def _pack_core_q(qc8):
    # fp8 [64, C, F2] -> [p, pair, k, i, f] -> [128, NPAIR*QCOLS]
    qc = qc8.reshape(NPAIR, 2, KT, 128, F2).transpose(3, 0, 2, 1, 4)
    return np.ascontiguousarray(qc).reshape(128, NPAIR * QCOLS)


def _prepare(query_features, labels, prototypes, indices):
    """Returns (in_maps, labels_i64, pn64)."""
    qf = np.asarray(query_features, dtype=np.float32).reshape(B, C, F2)
    labels = np.asarray(labels).astype(np.int64)
    protos = np.asarray(prototypes, dtype=np.float32)
    idx = np.asarray(indices).astype(np.int64)

    pg8 = protos[idx].astype(F8_NP)                      # [64, C] fp8
    pg = pg8.astype(np.float64)
    pn64 = np.sum(pg**2, axis=1)                         # matches device G
    negpn2_np = np.ascontiguousarray(
        np.concatenate([-pn64, -pn64]).reshape(128, 1).astype(np.float32)
    )
    pT_pack = np.ascontiguousarray(
        pg8.T.reshape(KT, 128, NWAY).transpose(1, 0, 2)
    ).reshape(128, KT * NWAY)
    bsel2_np = np.zeros((128, 2 * NPAIR - 1), dtype=BF16_NP)
    bsel2_np[0:NWAY, NPAIR - 1] = 1
    bsel2_np[NWAY:128, NPAIR] = 1

    qf8 = qf.astype(F8_NP)
    in_maps = [
        {
            "q": _pack_core_q(qf8[c * BPC : (c + 1) * BPC]),
            "pT": pT_pack,
            "negpn2": negpn2_np,
            "bsel2": bsel2_np,
        }
        for c in range(NCORES)
    ]
    return in_maps, labels, pn64


def kernel(query_features, labels, prototypes, indices, n_way):
    import time as _time

    t0 = _time.time()
    nc = _get_nc()
    t1 = _time.time()
    in_maps, labels, pn64 = _prepare(query_features, labels, prototypes, indices)
    t2 = _time.time()
    results = run_bass_kernel_spmd(nc, in_maps, list(range(NCORES))).results
    t3 = _time.time()
    print(
        f"[kernel] build={t1 - t0:.1f}s pack={t2 - t1:.1f}s run={t3 - t2:.1f}s",
        flush=True,
    )

    # Host-side finish: rsum[:, 0:32] holds per-image rowsums of G; image
    # local index l lives at row block 64*(l%4>=2)+class, column 2*(l//4)+(l%2).
    # rsum[0:28, 32] + rsum[0:4, 33] hold the per-pair-row sums of log s.
    larr = np.arange(BPC)
    rows0 = 64 * ((larr % 4) >= 2)
    cols = 2 * (larr // 4) + (larr % 2)
    total_lse = 0.0
    label_term = 0.0
    for c in range(NCORES):
        out = results[c]["rsum"].astype(np.float64)      # [128, 34]
        total_lse += float(
            out[0 : 2 * (CHAIN_A_LAST + 1), NPAIR].sum()
            + out[0 : 2 * (NU - 1 - CHAIN_A_LAST), NPAIR + 1].sum()
        )
        r2 = out[:, 0:NPAIR]
        lab = labels[c * BPC : (c + 1) * BPC]
        label_term += float(
            np.sum(2.0 * r2[rows0 + lab, cols] - F2 * pn64[lab])
        )
    loss = (total_lse - label_term) / (B * F2)
    return np.asarray(loss, dtype=np.float32)
